# revision 4
# baseline (speedup 1.0000x reference)
"""Trainium2 Bass kernel for nn_Cross_Attention (sparse attention variant).

Data-parallel over batch B=8 across 8 NeuronCores; each core computes one
batch element end to end:

  E2 = exp(x2)                       [N, D]   (key/query logits, shared exp)
  C~ = E2.T @ x1                     [D, D]   unnormalized context
  (key softmax denominator is a positive per-row scale of C~, so top-k
   selection runs directly on -C~; the scale folds into the exp that
   builds A = sum_i w_i softmax(masked C))
  agg~ = A @ E2.T                    [D, N]   (query softmax denominator is
                                               folded into LayerNorm eps)
  proj~ = agg~.T @ W^T               [N, 2D]
  out = LayerNorm(proj~ / rowsum) * ln_w + ln_b

Shapes hardcoded: B=8, N=4096, D=512 (H=W=64), 2D=1024.
"""

import sys

for p in ("/opt/pypackages", "/opt/trn_rl_repo"):
    if p not in sys.path:
        sys.path.insert(0, p)

import numpy as np  # noqa: E402

import concourse.bass as bass  # noqa: E402
import concourse.mybir as mybir  # noqa: E402
import concourse.tile as tile  # noqa: E402
from concourse import bacc  # noqa: E402
from concourse.bass_utils import run_bass_kernel_spmd  # noqa: E402
from concourse.masks import make_identity  # noqa: E402

B, N, D = 8, 4096, 512
O = 2 * D          # 1024
P = 128
NT = N // P        # 32 token tiles
DC = D // P        # 4 channel chunks
NCH = N // 512     # 8 column chunks of 512
TOPKS = [D // 2, (2 * D) // 3, (3 * D) // 4, (4 * D) // 5]  # 256, 341, 384, 409
EXCL = [D - k for k in TOPKS]  # excluded (bottom) counts: 256, 171, 128, 103
N_ITER = (max(EXCL) + 1 + 7) // 8  # 33 extraction iterations (need index 256)
NV = N_ITER * 8    # 264 scratch columns
LN_EPS = 1e-5
NEG_BIG = -1e30

F32 = mybir.dt.float32
F16 = mybir.dt.float16
AF = mybir.ActivationFunctionType
ALU = mybir.AluOpType
AX = mybir.AxisListType


def _bcast(ap, parts=P):
    """Broadcast a DRAM AP along a new leading partition dim."""
    return bass.AP(tensor=ap.tensor, offset=ap.offset, ap=[[0, parts]] + list(ap.ap))


def _build(fast_b, fast_ln):
    nc = bacc.Bacc(None, target_bir_lowering=False)

    x1 = nc.dram_tensor("x1", [N, D], F32, kind="ExternalInput")
    x2 = nc.dram_tensor("x2", [N, D], F32, kind="ExternalInput")
    wp = nc.dram_tensor("w_proj", [O, D], F32, kind="ExternalInput")
    aw = nc.dram_tensor("attn_w", [4], F32, kind="ExternalInput")
    bp = nc.dram_tensor("b_proj", [O], F32, kind="ExternalInput")
    lw = nc.dram_tensor("ln_w", [O], F32, kind="ExternalInput")
    lb = nc.dram_tensor("ln_b", [O], F32, kind="ExternalInput")
    out = nc.dram_tensor("out", [N, O], F32, kind="ExternalOutput")

    with tile.TileContext(nc) as tc:
        with (
            tc.tile_pool(name="persist", bufs=1) as pp,
            tc.tile_pool(name="stream", bufs=3) as sp,
            tc.tile_pool(name="big", bufs=1) as bigp,
            tc.tile_pool(name="mid", bufs=2) as midp,
            tc.tile_pool(name="aggp", bufs=1) as aggp,
        ):
            ident = pp.tile([P, P], F32, tag="ident")
            make_identity(nc, ident)
            identh = pp.tile([P, P], F16, tag="identh")
            nc.vector.tensor_copy(out=identh, in_=ident)
            wb = pp.tile([P, 4], F32, tag="wb")
            nc.sync.dma_start(out=wb, in_=_bcast(aw[:]))
            # per-token rowsum of exp(x2) (query softmax denominators), col nt
            rs_all = pp.tile([P, NT], F32, tag="rs_all")
            # E2^T: e2t[:, j, n] = exp(x2[n, j*128 + p]) for channel chunk j
            e2t = pp.tile([P, DC, N], F16, tag="e2t")

            # ---------------- Phase 1: exp, GEMM1 (context), E2^T ----------
            with tc.tile_pool(name="ps1", bufs=1, space="PSUM") as ps1, \
                 tc.tile_pool(name="ps1t", bufs=3, space="PSUM") as ps1t:
                cpsum = [ps1.tile([P, D], F32, tag=f"cp{m}", name=f"cp{m}") for m in range(DC)]
                for nt in range(NT):
                    x2t = sp.tile([P, D], F32, tag="x2t", name="x2t")
                    nc.sync.dma_start(out=x2t, in_=x2[nt * P:(nt + 1) * P, :])
                    e2 = sp.tile([P, D], F16, tag="e2", name="e2")
                    nc.scalar.activation(out=e2, in_=x2t, func=AF.Exp,
                                         accum_out=rs_all[:, nt:nt + 1])
                    x1t = sp.tile([P, D], F32, tag="x1t", name="x1t")
                    nc.gpsimd.dma_start(out=x1t, in_=x1[nt * P:(nt + 1) * P, :])
                    x1h = sp.tile([P, D], F16, tag="x1h", name="x1h")
                    nc.scalar.copy(out=x1h, in_=x1t)
                    for m in range(DC):
                        nc.tensor.matmul(cpsum[m], e2[:, m * P:(m + 1) * P], x1h,
                                         start=(nt == 0), stop=(nt == NT - 1))
                    tp = ps1t.tile([P, DC, P], F16, tag="tp1", name="tp1")
                    for j in range(DC):
                        nc.tensor.transpose(tp[:, j, :], e2[:, j * P:(j + 1) * P], identh)
                    nc.scalar.copy(out=e2t[:, :, nt * P:(nt + 1) * P], in_=tp)

                # evacuate GEMM1: negC~[m] = -cpsum[m] (unnormalized, negated)
                negc = [bigp.tile([P, D], F32, tag=f"negc{m}", name=f"negc{m}", bufs=2)
                        for m in range(DC)]
                for m in range(DC):
                    nc.scalar.mul(out=negc[m], in_=cpsum[m], mul=-1.0)

            # colsum[j][d] = sum_n E2T[d@j, n]  via ACT accumulate passes;
            # nivc = -1/colsum (the negated key-softmax inverse denominators)
            nivc = pp.tile([P, DC], F32, tag="nivc")
            junk = bigp.tile([P, N], F16, tag="junk", bufs=1)
            for j in range(DC):
                cs = midp.tile([P, 1], F32, tag="cs", name="cs")
                nc.scalar.activation(out=junk, in_=e2t[:, j, :], func=AF.Copy,
                                     accum_out=cs)
                nc.vector.tensor_scalar(out=nivc[:, j:j + 1], in0=cs, scalar1=-1.0,
                                        scalar2=None, op0=ALU.mult)
                nc.vector.reciprocal(out=nivc[:, j:j + 1], in_=nivc[:, j:j + 1])

            # W^T tiles (fp16) for GEMM3, built in extraction shadow
            wt = pp.tile([P, DC, O], F16, tag="wt")
            with tc.tile_pool(name="ps2t", bufs=2, space="PSUM") as ps2t:
                for ot in range(O // P):
                    wtile = sp.tile([P, D], F32, tag="wtile", name="wtile")
                    nc.sync.dma_start(out=wtile, in_=wp[ot * P:(ot + 1) * P, :])
                    tpw = ps2t.tile([P, DC, P], F32, tag="tpw", name="tpw")
                    for j in range(DC):
                        nc.tensor.transpose(tpw[:, j, :], wtile[:, j * P:(j + 1) * P], ident)
                    nc.scalar.copy(out=wt[:, :, ot * P:(ot + 1) * P], in_=tpw)

                # ---------------- Phase 2: extraction, A, GEMM2 ----------------
                agg = [[None] * NCH for _ in range(DC)]
                for m in range(DC):
                    nv = midp.tile([P, NV], F32, tag="nv", name="nv")
                    work = midp.tile([P, D], F32, tag="work", name="work", bufs=2)
                    nc.vector.max(out=nv[:, 0:8], in_=negc[m])
                    nc.vector.match_replace(out=work, in_to_replace=nv[:, 0:8],
                                            in_values=negc[m], imm_value=NEG_BIG)
                    for j in range(1, N_ITER):
                        nc.vector.max(out=nv[:, 8 * j:8 * j + 8], in_=work)
                        if j < N_ITER - 1:
                            nc.vector.match_replace(out=work, in_to_replace=nv[:, 8 * j:8 * j + 8],
                                                    in_values=work, imm_value=NEG_BIG)

                    rmin = midp.tile([P, 1], F32, tag="rmin", name="rmin")
                    nc.vector.tensor_reduce(out=rmin, in_=negc[m], axis=AX.X, op=ALU.min)
                    # E = exp(C - max C) = exp(nivc*negC~ - nivc*rmin)
                    ebias = midp.tile([P, 1], F32, tag="ebias", name="ebias")
                    nc.vector.tensor_scalar(out=ebias, in0=rmin, scalar1=nivc[:, m:m + 1],
                                            scalar2=-1.0, op0=ALU.mult, op1=ALU.mult)
                    ee = midp.tile([P, D], F32, tag="ee", name="ee", bufs=2)
                    sall = midp.tile([P, 1], F32, tag="sall", name="sall")
                    nc.scalar.activation(out=ee, in_=negc[m], func=AF.Exp,
                                         bias=ebias, scale=nivc[:, m:m + 1],
                                         accum_out=sall)
                    eex = midp.tile([P, NV], F32, tag="eex", name="eex")
                    nc.scalar.activation(out=eex, in_=nv, func=AF.Exp,
                                         bias=ebias, scale=nivc[:, m:m + 1])
                    pref = midp.tile([P, 4], F32, tag="pref", name="pref")
                    for i in range(4):
                        nc.vector.tensor_reduce(out=pref[:, i:i + 1], in_=eex[:, 0:EXCL[i]],
                                                axis=AX.XY, op=ALU.add)
                    # s_i = sall - pref_i ; c_i = w_i / s_i
                    svec = midp.tile([P, 4], F32, tag="svec", name="svec")
                    nc.vector.tensor_scalar(out=svec, in0=pref, scalar1=sall,
                                            scalar2=-1.0, op0=ALU.subtract, op1=ALU.mult)
                    sinv = midp.tile([P, 4], F32, tag="sinv", name="sinv")
                    nc.vector.reciprocal(out=sinv, in_=svec)
                    cvec = midp.tile([P, 4], F32, tag="cvec", name="cvec")
                    nc.vector.tensor_mul(out=cvec, in0=sinv, in1=wb)

                    # A = E * sum_i c_i * [negC~ <= t'_i]   (GPSIMD: DVE is busy)
                    gacc = midp.tile([P, D], F32, tag="gacc", name="gacc", bufs=1)
                    gtmp = midp.tile([P, D], F32, tag="gtmp", name="gtmp", bufs=1)
                    nc.gpsimd.tensor_scalar(out=gacc, in0=negc[m], scalar1=nv[:, EXCL[0]:EXCL[0] + 1],
                                            scalar2=cvec[:, 0:1], op0=ALU.is_le, op1=ALU.mult)
                    for i in range(1, 4):
                        nc.gpsimd.tensor_scalar(out=gtmp, in0=negc[m], scalar1=nv[:, EXCL[i]:EXCL[i] + 1],
                                                scalar2=cvec[:, i:i + 1], op0=ALU.is_le, op1=ALU.mult)
                        nc.gpsimd.tensor_tensor(out=gacc, in0=gacc, in1=gtmp, op=ALU.add)
                    amat = midp.tile([P, D], F32, tag="amat", name="amat", bufs=2)
                    nc.gpsimd.tensor_tensor(out=amat, in0=gacc, in1=ee, op=ALU.mult)

                    # A^T blocks: at[:, j, :] = (A[:, j*128:(j+1)*128]).T
                    at = midp.tile([P, DC, P], F16, tag="at", name="at", bufs=2)
                    tpa = ps2t.tile([P, DC, P], F32, tag="tpa", name="tpa")
                    for j in range(DC):
                        nc.tensor.transpose(tpa[:, j, :], amat[:, j * P:(j + 1) * P], ident)
                    nc.scalar.copy(out=at, in_=tpa)

                    # GEMM2 row m: agg~[m-chunk, :] = sum_j at_j.T @ e2t[j]
                    with tc.tile_pool(name=f"ps2g{m}", bufs=2, space="PSUM") as ps2g:
                        for nch in range(NCH):
                            g2 = ps2g.tile([P, 512], F32, tag="g2", name="g2")
                            for j in range(DC):
                                nc.tensor.matmul(g2, at[:, j, :], e2t[:, j, nch * 512:(nch + 1) * 512],
                                                 start=(j == 0), stop=(j == DC - 1))
                            ag = aggp.tile([P, 512], F16, tag=f"agg{m}_{nch}",
                                           name=f"agg{m}_{nch}")
                            nc.scalar.copy(out=ag, in_=g2)
                            agg[m][nch] = ag

            # ---------------- Phase 3: GEMM3 + LayerNorm ----------------
            # eps per token: LN_EPS * rowsum^2 (folds query-softmax normalizer)
            epsr = pp.tile([P, NT], F32, tag="epsr")
            if fast_b:
                nc.vector.tensor_mul(out=epsr, in0=rs_all, in1=rs_all)
                nc.vector.tensor_scalar_mul(epsr, epsr, LN_EPS)
            else:
                nc.vector.memset(epsr, LN_EPS)
            invrs = pp.tile([P, NT], F32, tag="invrs")
            nc.vector.reciprocal(out=invrs, in_=rs_all)

            if not fast_b:
                bb = pp.tile([P, O], F32, tag="bb")
                nc.sync.dma_start(out=bb, in_=_bcast(bp[:]))
            if not fast_ln:
                lwb = pp.tile([P, O], F32, tag="lwb")
                nc.sync.dma_start(out=lwb, in_=_bcast(lw[:]))
                lbb = pp.tile([P, O], F32, tag="lbb")
                nc.sync.dma_start(out=lbb, in_=_bcast(lb[:]))

            with tc.tile_pool(name="ps3", bufs=2, space="PSUM") as ps3:
                for nt in range(NT):
                    nch, col = nt // 4, (nt % 4) * P
                    ph = ps3.tile([P, O], F32, tag="ph", name="ph")
                    for oh in range(2):
                        for dt in range(DC):
                            nc.tensor.matmul(ph[:, oh * 512:(oh + 1) * 512],
                                             agg[dt][nch][:, col:col + P],
                                             wt[:, dt, oh * 512:(oh + 1) * 512],
                                             start=(dt == 0), stop=(dt == DC - 1))

                    if fast_b:
                        src = ph
                    else:
                        # proj = P~ * invrs + b  (general path, b_proj != 0)
                        src = sp.tile([P, O], F32, tag="tsb", name="tsb")
                        nc.vector.tensor_scalar(out=src, in0=ph, scalar1=invrs[:, nt:nt + 1],
                                                scalar2=None, op0=ALU.mult)
                        nc.vector.tensor_add(out=src, in0=src, in1=bb)

                    stats = sp.tile([P, 2, 6], F32, tag="stats", name="stats")
                    nc.vector.bn_stats(out=stats[:, 0, :], in_=src[:, 0:512])
                    nc.vector.bn_stats(out=stats[:, 1, :], in_=src[:, 512:1024])
                    mv = sp.tile([P, 2], F32, tag="mv", name="mv")
                    nc.vector.bn_aggr(out=mv, in_=stats)
                    sdv = sp.tile([P, 1], F32, tag="sdv", name="sdv")
                    nc.scalar.activation(out=sdv, in_=mv[:, 1:2], func=AF.Sqrt,
                                         bias=epsr[:, nt:nt + 1], scale=1.0)
                    rstd = sp.tile([P, 1], F32, tag="rstd", name="rstd")
                    nc.vector.reciprocal(out=rstd, in_=sdv)
                    nmr = sp.tile([P, 1], F32, tag="nmr", name="nmr")
                    nc.vector.tensor_scalar(out=nmr, in0=mv[:, 0:1], scalar1=rstd,
                                            scalar2=-1.0, op0=ALU.mult, op1=ALU.mult)
                    u = sp.tile([P, O], F32, tag="u", name="u")
                    nc.scalar.activation(out=u, in_=src, func=AF.Identity,
                                         bias=nmr, scale=rstd)
                    if not fast_ln:
                        nc.vector.tensor_mul(out=u, in0=u, in1=lwb)
                        nc.vector.tensor_add(out=u, in0=u, in1=lbb)
                    eng = nc.sync if nt % 2 == 0 else nc.gpsimd
                    eng.dma_start(out=out[nt * P:(nt + 1) * P, :], in_=u)

    nc.finalize()
    return nc


_NC_CACHE = {}


def kernel(x1, x2, W_proj, b_proj, ln_w, ln_b, attn_w, H=64, W=64):
    x1 = np.ascontiguousarray(np.asarray(x1, np.float32))
    x2 = np.ascontiguousarray(np.asarray(x2, np.float32))
    W_proj = np.ascontiguousarray(np.asarray(W_proj, np.float32))
    b_proj = np.ascontiguousarray(np.asarray(b_proj, np.float32))
    ln_w = np.ascontiguousarray(np.asarray(ln_w, np.float32))
    ln_b = np.ascontiguousarray(np.asarray(ln_b, np.float32))
    attn_w = np.ascontiguousarray(np.asarray(attn_w, np.float32))

    fast_b = bool(np.all(b_proj == 0.0))
    fast_ln = bool(np.all(ln_w == 1.0) and np.all(ln_b == 0.0))
    key = (fast_b, fast_ln)
    if key not in _NC_CACHE:
        _NC_CACHE[key] = _build(fast_b, fast_ln)
    nc = _NC_CACHE[key]

    in_maps = [
        {"x1": x1[b], "x2": x2[b], "w_proj": W_proj, "attn_w": attn_w,
         "b_proj": b_proj, "ln_w": ln_w, "ln_b": ln_b}
        for b in range(B)
    ]
    res = run_bass_kernel_spmd(nc, in_maps, core_ids=list(range(B)))
    return np.stack([res.results[b]["out"] for b in range(B)], axis=0)


# revision 11
# speedup vs baseline: 15429.0270x; 15429.0270x over previous
"""Trainium2 Bass kernel for nn_Cross_Attention (sparse attention variant).

Data-parallel over batch B=8 across 8 NeuronCores; each core computes one
batch element end to end:

  E2 = exp(x2)                       [N, D]   (key/query logits, shared exp)
  C~ = E2.T @ x1                     [D, D]   unnormalized context
  (key softmax denominator is a positive per-row scale of C~, so top-k
   selection runs directly on -C~; the scale folds into the exp that
   builds A = sum_i w_i softmax(masked C))
  agg~ = A @ E2.T                    [D, N]   (query softmax denominator is
                                               folded into LayerNorm eps)
  proj~ = agg~.T @ W^T               [N, 2D]
  out = LayerNorm(proj~ / rowsum) * ln_w + ln_b

Shapes hardcoded: B=8, N=4096, D=512 (H=W=64), 2D=1024.
"""

import sys

for p in ("/opt/pypackages", "/opt/trn_rl_repo"):
    if p not in sys.path:
        sys.path.insert(0, p)

import numpy as np  # noqa: E402

import concourse.bass as bass  # noqa: E402
import concourse.mybir as mybir  # noqa: E402
import concourse.tile as tile  # noqa: E402
from concourse import bacc  # noqa: E402
from concourse.bass_utils import run_bass_kernel_spmd  # noqa: E402
from concourse.masks import make_identity  # noqa: E402

B, N, D = 8, 4096, 512
O = 2 * D          # 1024
P = 128
NT = N // P        # 32 token tiles
DC = D // P        # 4 channel chunks
NCH = N // 512     # 8 column chunks of 512
TOPKS = [D // 2, (2 * D) // 3, (3 * D) // 4, (4 * D) // 5]  # 256, 341, 384, 409
EXCL = [D - k for k in TOPKS]  # excluded (bottom) counts: 256, 171, 128, 103
N_ITER = (max(EXCL) + 1 + 7) // 8  # 33 extraction iterations (need index 256)
NV = N_ITER * 8    # 264 scratch columns
LN_EPS = 1e-5
NEG_BIG = -1e30

F32 = mybir.dt.float32
F16 = mybir.dt.float16
AF = mybir.ActivationFunctionType
ALU = mybir.AluOpType
AX = mybir.AxisListType


def _bcast(ap, parts=P):
    """Broadcast a DRAM AP along a new leading partition dim."""
    return bass.AP(tensor=ap.tensor, offset=ap.offset, ap=[[0, parts]] + list(ap.ap))


def _build(fast_b, fast_ln):
    nc = bacc.Bacc(None, target_bir_lowering=False)

    x1 = nc.dram_tensor("x1", [N, D], F32, kind="ExternalInput")
    x2 = nc.dram_tensor("x2", [N, D], F32, kind="ExternalInput")
    wp = nc.dram_tensor("w_proj", [O, D], F32, kind="ExternalInput")
    aw = nc.dram_tensor("attn_w", [4], F32, kind="ExternalInput")
    bp = nc.dram_tensor("b_proj", [O], F32, kind="ExternalInput")
    lw = nc.dram_tensor("ln_w", [O], F32, kind="ExternalInput")
    lb = nc.dram_tensor("ln_b", [O], F32, kind="ExternalInput")
    out = nc.dram_tensor("out", [N, O], F32, kind="ExternalOutput")

    with tile.TileContext(nc) as tc:
        with (
            tc.tile_pool(name="persist", bufs=1) as pp,
            tc.tile_pool(name="stream", bufs=3) as sp,
            tc.tile_pool(name="big", bufs=1) as bigp,
            tc.tile_pool(name="mid", bufs=2) as midp,
            tc.tile_pool(name="aggp", bufs=1) as aggp,
        ):
            ident = pp.tile([P, P], F32, tag="ident")
            make_identity(nc, ident)
            identh = pp.tile([P, P], F16, tag="identh")
            nc.vector.tensor_copy(out=identh, in_=ident)
            wb = pp.tile([P, 4], F32, tag="wb")
            nc.sync.dma_start(out=wb, in_=_bcast(aw[:]))
            # per-token rowsum of exp(x2) (query softmax denominators), col nt
            rs_all = pp.tile([P, NT], F32, tag="rs_all")
            # E2^T: e2t[:, j, n] = exp(x2[n, j*128 + p]) for channel chunk j
            e2t = pp.tile([P, DC, N], F16, tag="e2t")

            # ---------------- Phase 1: exp, GEMM1 (context), E2^T ----------
            with tc.tile_pool(name="ps1", bufs=1, space="PSUM") as ps1, \
                 tc.tile_pool(name="ps1t", bufs=3, space="PSUM") as ps1t, \
                 tc.tile_pool(name="p1", bufs=3) as p1:
                cpsum = [ps1.tile([P, D], F32, tag=f"cp{m}", name=f"cp{m}") for m in range(DC)]
                for ntg in range(NT // 4):
                    x2s = p1.tile([P, 4, D], F32, tag="x2s", name="x2s", bufs=2)
                    nc.sync.dma_start(
                        out=x2s,
                        in_=x2[ntg * 4 * P:(ntg + 1) * 4 * P, :].rearrange("(a p) d -> p a d", p=P))
                    x1s = p1.tile([P, 4, D], F32, tag="x1s", name="x1s", bufs=2)
                    nc.gpsimd.dma_start(
                        out=x1s,
                        in_=x1[ntg * 4 * P:(ntg + 1) * 4 * P, :].rearrange("(a p) d -> p a d", p=P))
                    for a in range(4):
                        nt = ntg * 4 + a
                        e2 = p1.tile([P, D], F16, tag="e2", name="e2")
                        nc.scalar.activation(out=e2, in_=x2s[:, a, :], func=AF.Exp,
                                             accum_out=rs_all[:, nt:nt + 1])
                        x1h = p1.tile([P, D], F16, tag="x1h", name="x1h")
                        if nt % 2 == 0:
                            nc.scalar.copy(out=x1h, in_=x1s[:, a, :])
                        else:
                            nc.gpsimd.tensor_copy(out=x1h, in_=x1s[:, a, :])
                        for m in range(DC):
                            nc.tensor.matmul(cpsum[m], e2[:, m * P:(m + 1) * P], x1h,
                                             start=(nt == 0), stop=(nt == NT - 1))
                        tp = ps1t.tile([P, DC, P], F16, tag="tp1", name="tp1")
                        for j in range(DC):
                            nc.tensor.transpose(tp[:, j, :], e2[:, j * P:(j + 1) * P], identh)
                        nc.scalar.copy(out=e2t[:, :, nt * P:(nt + 1) * P], in_=tp)

                # evacuate GEMM1: negC~[m] = -cpsum[m] (unnormalized, negated)
                negc = [bigp.tile([P, D], F32, tag=f"negc{m}", name=f"negc{m}", bufs=2)
                        for m in range(DC)]
                for m in range(DC):
                    nc.scalar.mul(out=negc[m], in_=cpsum[m], mul=-1.0)

            # colsum[j][d] = sum_n E2T[d@j, n]  via ACT accumulate passes;
            # nivc = -1/colsum (the negated key-softmax inverse denominators)
            nivc = pp.tile([P, DC], F32, tag="nivc")
            junk = bigp.tile([P, N], F16, tag="junk", bufs=1)
            for j in range(DC):
                cs = midp.tile([P, 1], F32, tag="cs", name="cs")
                nc.scalar.activation(out=junk, in_=e2t[:, j, :], func=AF.Copy,
                                     accum_out=cs)
                nc.vector.tensor_scalar(out=nivc[:, j:j + 1], in0=cs, scalar1=-1.0,
                                        scalar2=None, op0=ALU.mult)
                nc.vector.reciprocal(out=nivc[:, j:j + 1], in_=nivc[:, j:j + 1])

            # W^T tiles (fp16) for GEMM3, built in extraction shadow
            wt = pp.tile([P, DC, O], F16, tag="wt")
            with tc.tile_pool(name="ps2t", bufs=2, space="PSUM") as ps2t:
                for og in range(2):
                    ws = midp.tile([P, 4, D], F32, tag="ws", name="ws", bufs=1)
                    nc.sync.dma_start(
                        out=ws,
                        in_=wp[og * 4 * P:(og + 1) * 4 * P, :].rearrange("(a p) d -> p a d", p=P))
                    for a in range(4):
                        ot = og * 4 + a
                        tpw = ps2t.tile([P, DC, P], F32, tag="tpw", name="tpw")
                        for j in range(DC):
                            nc.tensor.transpose(tpw[:, j, :], ws[:, a, j * P:(j + 1) * P], ident)
                        nc.scalar.copy(out=wt[:, :, ot * P:(ot + 1) * P], in_=tpw)

                # ---------------- Phase 2: extraction, A, GEMM2 ----------------
                agg = [[None] * NCH for _ in range(DC)]
                for m in range(DC):
                    nv = midp.tile([P, NV], F32, tag="nv", name="nv")
                    work = midp.tile([P, D], F32, tag="work", name="work", bufs=2)
                    nc.vector.max(out=nv[:, 0:8], in_=negc[m])
                    nc.vector.match_replace(out=work, in_to_replace=nv[:, 0:8],
                                            in_values=negc[m], imm_value=NEG_BIG)
                    for j in range(1, N_ITER):
                        nc.vector.max(out=nv[:, 8 * j:8 * j + 8], in_=work)
                        if j < N_ITER - 1:
                            nc.vector.match_replace(out=work, in_to_replace=nv[:, 8 * j:8 * j + 8],
                                                    in_values=work, imm_value=NEG_BIG)

                    rmin = midp.tile([P, 1], F32, tag="rmin", name="rmin")
                    nc.vector.tensor_reduce(out=rmin, in_=negc[m], axis=AX.X, op=ALU.min)
                    # E = exp(C - max C) = exp(nivc*negC~ - nivc*rmin)
                    ebias = midp.tile([P, 1], F32, tag="ebias", name="ebias")
                    nc.vector.tensor_scalar(out=ebias, in0=rmin, scalar1=nivc[:, m:m + 1],
                                            scalar2=-1.0, op0=ALU.mult, op1=ALU.mult)
                    ee = midp.tile([P, D], F32, tag="ee", name="ee", bufs=2)
                    sall = midp.tile([P, 1], F32, tag="sall", name="sall")
                    nc.scalar.activation(out=ee, in_=negc[m], func=AF.Exp,
                                         bias=ebias, scale=nivc[:, m:m + 1],
                                         accum_out=sall)
                    eex = midp.tile([P, NV], F32, tag="eex", name="eex")
                    nc.scalar.activation(out=eex, in_=nv, func=AF.Exp,
                                         bias=ebias, scale=nivc[:, m:m + 1])
                    pref = midp.tile([P, 4], F32, tag="pref", name="pref")
                    for i in range(4):
                        nc.vector.tensor_reduce(out=pref[:, i:i + 1], in_=eex[:, 0:EXCL[i]],
                                                axis=AX.XY, op=ALU.add)
                    # s_i = sall - pref_i ; c_i = w_i / s_i
                    svec = midp.tile([P, 4], F32, tag="svec", name="svec")
                    nc.vector.tensor_scalar(out=svec, in0=pref, scalar1=sall,
                                            scalar2=-1.0, op0=ALU.subtract, op1=ALU.mult)
                    sinv = midp.tile([P, 4], F32, tag="sinv", name="sinv")
                    nc.vector.reciprocal(out=sinv, in_=svec)
                    cvec = midp.tile([P, 4], F32, tag="cvec", name="cvec")
                    nc.vector.tensor_mul(out=cvec, in0=sinv, in1=wb)

                    # A = E * sum_i c_i * [negC~ <= t'_i]   (GPSIMD: DVE is busy)
                    gacc = midp.tile([P, D], F32, tag="gacc", name="gacc", bufs=1)
                    gtmp = midp.tile([P, D], F32, tag="gtmp", name="gtmp", bufs=1)
                    nc.gpsimd.tensor_scalar(out=gacc, in0=negc[m], scalar1=nv[:, EXCL[0]:EXCL[0] + 1],
                                            scalar2=cvec[:, 0:1], op0=ALU.is_le, op1=ALU.mult)
                    for i in range(1, 4):
                        nc.gpsimd.tensor_scalar(out=gtmp, in0=negc[m], scalar1=nv[:, EXCL[i]:EXCL[i] + 1],
                                                scalar2=cvec[:, i:i + 1], op0=ALU.is_le, op1=ALU.mult)
                        nc.gpsimd.tensor_tensor(out=gacc, in0=gacc, in1=gtmp, op=ALU.add)
                    amat = midp.tile([P, D], F32, tag="amat", name="amat", bufs=2)
                    nc.gpsimd.tensor_tensor(out=amat, in0=gacc, in1=ee, op=ALU.mult)

                    # A^T blocks: at[:, j, :] = (A[:, j*128:(j+1)*128]).T
                    at = midp.tile([P, DC, P], F16, tag="at", name="at", bufs=2)
                    tpa = ps2t.tile([P, DC, P], F32, tag="tpa", name="tpa")
                    for j in range(DC):
                        nc.tensor.transpose(tpa[:, j, :], amat[:, j * P:(j + 1) * P], ident)
                    nc.scalar.copy(out=at, in_=tpa)

                    # GEMM2 row m: agg~[m-chunk, :] = sum_j at_j.T @ e2t[j]
                    with tc.tile_pool(name=f"ps2g{m}", bufs=2, space="PSUM") as ps2g:
                        for nch in range(NCH):
                            g2 = ps2g.tile([P, 512], F32, tag="g2", name="g2")
                            for j in range(DC):
                                nc.tensor.matmul(g2, at[:, j, :], e2t[:, j, nch * 512:(nch + 1) * 512],
                                                 start=(j == 0), stop=(j == DC - 1))
                            ag = aggp.tile([P, 512], F16, tag=f"agg{m}_{nch}",
                                           name=f"agg{m}_{nch}")
                            nc.scalar.copy(out=ag, in_=g2)
                            agg[m][nch] = ag

            # ---------------- Phase 3: GEMM3 + LayerNorm ----------------
            # eps per token: LN_EPS * rowsum^2 (folds query-softmax normalizer)
            epsr = pp.tile([P, NT], F32, tag="epsr")
            if fast_b:
                nc.vector.tensor_mul(out=epsr, in0=rs_all, in1=rs_all)
                nc.vector.tensor_scalar_mul(epsr, epsr, LN_EPS)
            else:
                nc.vector.memset(epsr, LN_EPS)
            invrs = pp.tile([P, NT], F32, tag="invrs")
            nc.vector.reciprocal(out=invrs, in_=rs_all)

            if not fast_b:
                bb = pp.tile([P, O], F32, tag="bb")
                nc.sync.dma_start(out=bb, in_=_bcast(bp[:]))
            if not fast_ln:
                lwb = pp.tile([P, O], F32, tag="lwb")
                nc.sync.dma_start(out=lwb, in_=_bcast(lw[:]))
                lbb = pp.tile([P, O], F32, tag="lbb")
                nc.sync.dma_start(out=lbb, in_=_bcast(lb[:]))

            # column sums of W^T per d-chunk (for the LayerNorm mean column)
            if fast_b:
                wsf = pp.tile([P, DC], F32, tag="wsf")
                for dt in range(DC):
                    nc.vector.tensor_reduce(out=wsf[:, dt:dt + 1], in_=wt[:, dt, :],
                                            axis=AX.X, op=ALU.add)
                wsum = pp.tile([P, DC], F16, tag="wsum")
                nc.vector.tensor_copy(out=wsum, in_=wsf)

            with tc.tile_pool(name="ps3", bufs=2, space="PSUM") as ps3, \
                 tc.tile_pool(name="ps3m", bufs=2, space="PSUM") as ps3m:
                usup = None
                for nt in range(NT):
                    nch, col = nt // 4, (nt % 4) * P
                    if nt % 4 == 0:
                        usup = sp.tile([P, 4, O], F32, tag="usup", name="usup", bufs=2)
                    ph = ps3.tile([P, O], F32, tag="ph", name="ph")
                    for oh in range(2):
                        for dt in range(DC):
                            nc.tensor.matmul(ph[:, oh * 512:(oh + 1) * 512],
                                             agg[dt][nch][:, col:col + P],
                                             wt[:, dt, oh * 512:(oh + 1) * 512],
                                             start=(dt == 0), stop=(dt == DC - 1))

                    if fast_b:
                        # rowsum of proj~ via an extra 1-wide matmul column
                        phm = ps3m.tile([P, 8], F32, tag="phm", name="phm")
                        for dt in range(DC):
                            nc.tensor.matmul(phm[:, 0:1],
                                             agg[dt][nch][:, col:col + P],
                                             wsum[:, dt:dt + 1],
                                             start=(dt == 0), stop=(dt == DC - 1))
                        # sum of squares via ACT Square + accumulate
                        junk3 = sp.tile([P, O], F16, tag="junk3", name="junk3")
                        ssq = sp.tile([P, 1], F32, tag="ssq", name="ssq")
                        nc.scalar.activation(out=junk3, in_=ph, func=AF.Square,
                                             accum_out=ssq)
                        mean = sp.tile([P, 1], F32, tag="mean", name="mean")
                        nc.vector.tensor_scalar(out=mean, in0=phm[:, 0:1], scalar1=1.0 / O,
                                                scalar2=None, op0=ALU.mult)
                        m2 = sp.tile([P, 1], F32, tag="m2", name="m2")
                        nc.vector.tensor_mul(out=m2, in0=mean, in1=mean)
                        # bias3 = epsr - mean^2 ; var + eps = ssq/O + bias3
                        bias3 = sp.tile([P, 1], F32, tag="bias3", name="bias3")
                        nc.vector.tensor_scalar(out=bias3, in0=m2, scalar1=-1.0,
                                                scalar2=epsr[:, nt:nt + 1],
                                                op0=ALU.mult, op1=ALU.add)
                        sdv = sp.tile([P, 1], F32, tag="sdv", name="sdv")
                        nc.scalar.activation(out=sdv, in_=ssq, func=AF.Sqrt,
                                             bias=bias3, scale=1.0 / O)
                        src = ph
                    else:
                        # proj = P~ * invrs + b  (general path, b_proj != 0)
                        src = sp.tile([P, O], F32, tag="tsb", name="tsb")
                        nc.vector.tensor_scalar(out=src, in0=ph, scalar1=invrs[:, nt:nt + 1],
                                                scalar2=None, op0=ALU.mult)
                        nc.vector.tensor_add(out=src, in0=src, in1=bb)
                        stats = sp.tile([P, 2, 6], F32, tag="stats", name="stats")
                        nc.vector.bn_stats(out=stats[:, 0, :], in_=src[:, 0:512])
                        nc.vector.bn_stats(out=stats[:, 1, :], in_=src[:, 512:1024])
                        mv = sp.tile([P, 2], F32, tag="mv", name="mv")
                        nc.vector.bn_aggr(out=mv, in_=stats)
                        mean = mv[:, 0:1]
                        sdv = sp.tile([P, 1], F32, tag="sdv", name="sdv")
                        nc.scalar.activation(out=sdv, in_=mv[:, 1:2], func=AF.Sqrt,
                                             bias=epsr[:, nt:nt + 1], scale=1.0)

                    rstd = sp.tile([P, 1], F32, tag="rstd", name="rstd")
                    nc.vector.reciprocal(out=rstd, in_=sdv)
                    nmr = sp.tile([P, 1], F32, tag="nmr", name="nmr")
                    nc.vector.tensor_scalar(out=nmr, in0=mean, scalar1=rstd,
                                            scalar2=-1.0, op0=ALU.mult, op1=ALU.mult)
                    u = usup[:, nt % 4, :]
                    nc.scalar.activation(out=u, in_=src, func=AF.Identity,
                                         bias=nmr, scale=rstd)
                    if not fast_ln:
                        nc.vector.tensor_mul(out=u, in0=u, in1=lwb)
                        nc.vector.tensor_add(out=u, in0=u, in1=lbb)
                    if nt % 4 == 3:
                        eng = nc.sync if (nt // 4) % 2 == 0 else nc.gpsimd
                        eng.dma_start(
                            out=out[(nt - 3) * P:(nt + 1) * P, :].rearrange("(a p) o -> p a o", p=P),
                            in_=usup)

    nc.finalize()
    return nc


_NC_CACHE = {}


def kernel(x1, x2, W_proj, b_proj, ln_w, ln_b, attn_w, H=64, W=64):
    x1 = np.ascontiguousarray(np.asarray(x1, np.float32))
    x2 = np.ascontiguousarray(np.asarray(x2, np.float32))
    W_proj = np.ascontiguousarray(np.asarray(W_proj, np.float32))
    b_proj = np.ascontiguousarray(np.asarray(b_proj, np.float32))
    ln_w = np.ascontiguousarray(np.asarray(ln_w, np.float32))
    ln_b = np.ascontiguousarray(np.asarray(ln_b, np.float32))
    attn_w = np.ascontiguousarray(np.asarray(attn_w, np.float32))

    fast_b = bool(np.all(b_proj == 0.0))
    fast_ln = bool(np.all(ln_w == 1.0) and np.all(ln_b == 0.0))
    key = (fast_b, fast_ln)
    if key not in _NC_CACHE:
        _NC_CACHE[key] = _build(fast_b, fast_ln)
    nc = _NC_CACHE[key]

    in_maps = [
        {"x1": x1[b], "x2": x2[b], "w_proj": W_proj, "attn_w": attn_w,
         "b_proj": b_proj, "ln_w": ln_w, "ln_b": ln_b}
        for b in range(B)
    ]
    res = run_bass_kernel_spmd(nc, in_maps, core_ids=list(range(B)))
    return np.stack([res.results[b]["out"] for b in range(B)], axis=0)
